# revision 6
# baseline (speedup 1.0000x reference)
"""Trainium2 Bass kernel for the CgpHmm scaled-forward layer.

Computes loglik[b] = scaled HMM forward log-likelihood over B=128 sequences
of length T=8192 with S=128 hidden states and an alphabet of E=6 symbols.

Strategy: data-parallel over batch (16 seqs/core on 8 cores) PLUS
speculative time-segmentation to break the sequential scan (segments
start from a uniform vector and re-run the last W=3 steps of the previous
segment as warmup; per-sequence loglik telescopes into sums of
log(colsum) captured at warmup end and segment end, assembled on host in
f64; emissions pre-divided by f_sym so no on-device renormalization).

ENGINE LAYOUT (the optimization target; DVE was the 195us baseline's
bottleneck at ~1.04ns/col for the emission multiply, all chains):

  - chains d0,d1 (480 cols each, 30 segs x 91 real steps): classic
    mm (PE) -> tensor_mul (DVE, PSUM f32 x em fp8 -> bf16), every link.
  - tracks tA,tB (496 cols each, 31 segs x 44 real steps): each advances
    every OTHER link, so the 3-stage serial path
    mm (PE) -> copy PSUM->SBUF bf16 (Act) -> tensor_mul (Pool, SBUF)
    (~2.5us) fits inside two ~1.33us link periods.  Per link this adds
    only ONE Act copy + ONE Pool multiply, so Act/Pool offload 496 of
    1456 columns per link from DVE.  (gpsimd has no PSUM port, hence the
    Act bounce; a single combined Act+Pool chain advancing every link
    was measured at 267us -- the serial path head-of-line-blocks the
    in-order PE queue.)

Coverage: seg0 covers steps 1..94, d-segs 1..59 cover 91 steps each,
pool segs cover 44 each: 94 + 59*91 + 62*44 = 8191 = T-1.  94 links.

Predicted period ~1.33us/link (DVE 2x665ns) -> ~130us incl preamble.
Emissions host-gathered to fp8e4, double-buffer streamed from HBM.
"""

import sys

import numpy as np

sys.path.insert(0, "/opt/trn_rl_repo")

P = 128          # states / partitions
BL = 16          # sequences per core
N_CORES = 8
B_FULL = 128
T_FULL = 8192
E_SYM = 6

W_WARM = 3
SEGS_D = 30                   # segments per DVE chain (x2 chains)
SEGS_T = 31                   # segments per pool track (x2 tracks)
L_D = 91                      # real steps per DVE segment (seg0: L_D+W)
L_T = 44                      # real steps per pool segment
LINKS = L_D + W_WARM          # 94 global links (DVE chains act each link)
TLINKS = L_T + W_WARM         # 47 track links (tracks alternate links)
assert 2 * TLINKS == LINKS
N_DSEG = 2 * SEGS_D           # 60
N_TSEG = 2 * SEGS_T           # 62
assert (L_D + W_WARM) + (N_DSEG - 1) * L_D + N_TSEG * L_T == T_FULL - 1
W_D = SEGS_D * BL             # 480 cols per DVE chain
W_T = SEGS_T * BL             # 496 cols per pool track
assert W_D * 4 <= 2048 and W_T * 4 <= 2048   # psum bank cap

# boundary between d-seg coverage and pool-seg coverage (first pool step)
T_POOL0 = 1 + (L_D + W_WARM) + (N_DSEG - 1) * L_D    # 5464

N_ABUF = 3
# em DMA chunks, in the consumer's own link units
CB_D = [0, 5]
while CB_D[-1] < LINKS:
    CB_D.append(min(LINKS, CB_D[-1] + 17))
CB_T = [0, 4]
while CB_T[-1] < TLINKS:
    CB_T.append(min(TLINKS, CB_T[-1] + 9))
N_DMA_SLICES = 4
MM_DTYPE = "bfloat16"
EM_DTYPE = "float8e4"


def _dseg_t0(s):
    """Warmup-start step of DVE segment s (0..59)."""
    if s == 0:
        return 1                            # covers steps 1..94, no warmup
    return 1 + (L_D + W_WARM) + (s - 1) * L_D - W_WARM


def _tseg_t0(p):
    """Warmup-start step of pool segment p (0..61)."""
    return T_POOL0 + p * L_T - W_WARM


def build_nc(debug=False):
    import concourse.bacc as bacc
    import concourse.bass as bass  # noqa: F401
    import concourse.mybir as mybir
    import concourse.tile as tile

    nc = bacc.Bacc(None, target_bir_lowering=False, debug=debug)

    f32 = mybir.dt.float32
    mdt = getattr(mybir.dt, MM_DTYPE)
    edt = getattr(mybir.dt, EM_DTYPE)

    em_d = [nc.dram_tensor(f"emd{j}", [P, LINKS * W_D], edt,
                           kind="ExternalInput") for j in range(2)]
    em_t = [nc.dram_tensor(f"emt{j}", [P, TLINKS * W_T], edt,
                           kind="ExternalInput") for j in range(2)]
    a_d = nc.dram_tensor("amat", [P, P], mdt, kind="ExternalInput")
    aid = [nc.dram_tensor(f"ainitd{j}", [P, W_D], mdt,
                          kind="ExternalInput") for j in range(2)]
    ait = [nc.dram_tensor(f"ainitt{j}", [P, W_T], mdt,
                          kind="ExternalInput") for j in range(2)]
    afd = [nc.dram_tensor(f"afind{j}", [P, W_D], f32,
                          kind="ExternalOutput") for j in range(2)]
    aft = [nc.dram_tensor(f"afint{j}", [P, W_T], f32,
                          kind="ExternalOutput") for j in range(2)]
    std = [nc.dram_tensor(f"stashd{j}", [P, W_D], f32,
                          kind="ExternalOutput") for j in range(2)]
    stt = [nc.dram_tensor(f"stasht{j}", [P, W_T], f32,
                          kind="ExternalOutput") for j in range(2)]

    def chunk_dma(emb_tile, em_dram, bounds, k, w):
        l0, l1 = bounds[k], bounds[k + 1]
        cols_k = (l1 - l0) * w
        base = l0 * w
        per = (cols_k + N_DMA_SLICES - 1) // N_DMA_SLICES
        for s in range(N_DMA_SLICES):
            o0 = s * per
            o1 = min(cols_k, o0 + per)
            if o0 >= o1:
                break
            nc.sync.dma_start(emb_tile[:, o0:o1],
                              em_dram[:, base + o0:base + o1])

    cmax_d = max(b - a for a, b in zip(CB_D, CB_D[1:]))
    cmax_t = max(b - a for a, b in zip(CB_T, CB_T[1:]))

    with tile.TileContext(nc) as tc, \
            tc.tile_pool(name="sb", bufs=1) as sbp, \
            tc.tile_pool(name="ps", bufs=1, space="PSUM") as psp:
        a_sb = sbp.tile([P, P], mdt, name="a_sb")
        ald = [[sbp.tile([P, W_D], mdt, name=f"ald{j}_{k}")
                for k in range(N_ABUF)] for j in range(2)]
        alt = [[sbp.tile([P, W_T], mdt, name=f"alt{j}_{k}")
                for k in range(N_ABUF)] for j in range(2)]
        embd = [[sbp.tile([P, cmax_d * W_D], edt, name=f"embd{j}_{k}")
                 for k in range(2)] for j in range(2)]
        embt = [[sbp.tile([P, cmax_t * W_T], edt, name=f"embt{j}_{k}")
                 for k in range(2)] for j in range(2)]
        tmp = [[sbp.tile([P, W_T], mdt, name=f"tmp{j}_{k}")
                for k in range(2)] for j in range(2)]
        stash_d_sb = [sbp.tile([P, W_D], f32, name=f"stshd{j}")
                      for j in range(2)]
        stash_t_sb = [sbp.tile([P, W_T], f32, name=f"stsht{j}")
                      for j in range(2)]
        fin_d = [sbp.tile([P, W_D], f32, name=f"find{j}") for j in range(2)]
        fin_t = [sbp.tile([P, W_T], f32, name=f"fint{j}") for j in range(2)]
        psd = [[psp.tile([P, W_D], f32, name=f"psd{j}_{k}")
                for k in range(2)] for j in range(2)]
        pst = [[psp.tile([P, W_T], f32, name=f"pst{j}_{k}")
                for k in range(2)] for j in range(2)]

        # preamble loads
        nc.sync.dma_start(a_sb[:], a_d[:])
        for j in range(2):
            nc.sync.dma_start(ald[j][0][:], aid[j][:])
            nc.sync.dma_start(alt[j][0][:], ait[j][:])
            chunk_dma(embd[j][0], em_d[j], CB_D, 0, W_D)
            chunk_dma(embt[j][0], em_t[j], CB_T, 0, W_T)

        # load A as the PE stationary operand (result discarded)
        nc.tensor.matmul(psd[0][0][:], a_sb[:], ald[0][0][:])

        import bisect
        for l in range(LINKS):
            kd = bisect.bisect_right(CB_D, l) - 1
            if l == CB_D[kd] and kd + 1 < len(CB_D) - 1:
                for j in range(2):
                    chunk_dma(embd[j][(kd + 1) % 2], em_d[j], CB_D,
                              kd + 1, W_D)
            # DVE chains, every link
            for j in range(2):
                c0 = (l - CB_D[kd]) * W_D
                cur = ald[j][l % N_ABUF]
                nxt = ald[j][(l + 1) % N_ABUF]
                pp = psd[j][l % 2][:]
                nc.tensor.matmul(pp, a_sb[:], cur[:])
                nc.vector.tensor_mul(nxt[:], pp,
                                     embd[j][kd % 2][:, c0:c0 + W_D])
                if l == W_WARM - 1:
                    nc.scalar.copy(stash_d_sb[j][:], nxt[:])
            # pool track (alternating), 3-stage path spread over 2 links
            j = l % 2
            tl = l // 2
            kt = bisect.bisect_right(CB_T, tl) - 1
            if tl == CB_T[kt] and j == 0 and kt + 1 < len(CB_T) - 1:
                for jj in range(2):
                    chunk_dma(embt[jj][(kt + 1) % 2], em_t[jj], CB_T,
                              kt + 1, W_T)
            c0 = (tl - CB_T[kt]) * W_T
            cur = alt[j][tl % N_ABUF]
            nxt = alt[j][(tl + 1) % N_ABUF]
            pp = pst[j][tl % 2][:]
            tm = tmp[j][tl % 2]
            nc.tensor.matmul(pp, a_sb[:], cur[:])
            nc.scalar.copy(tm[:], pp)
            nc.gpsimd.tensor_mul(nxt[:], tm[:],
                                 embt[j][kt % 2][:, c0:c0 + W_T])
            if tl == W_WARM - 1:
                nc.scalar.copy(stash_t_sb[j][:], nxt[:])

        for j in range(2):
            nc.scalar.copy(fin_d[j][:], ald[j][LINKS % N_ABUF][:])
            nc.scalar.copy(fin_t[j][:], alt[j][TLINKS % N_ABUF][:])
            nc.sync.dma_start(std[j][:], stash_d_sb[j][:])
            nc.sync.dma_start(stt[j][:], stash_t_sb[j][:])
            nc.sync.dma_start(afd[j][:], fin_d[j][:])
            nc.sync.dma_start(aft[j][:], fin_t[j][:])

    # A never changes: strip all but the first ldweights so matmuls reuse
    # the resident PE array.
    seen_ldw = False
    for f in nc.m.functions:
        for b in f.blocks:
            new = []
            for ins in b.instructions:
                if isinstance(ins, mybir.InstLdweights):
                    si = ins.sync_info
                    has_sync = si is not None and (
                        len(si.on_wait or []) or len(si.on_update or []))
                    if seen_ldw and not has_sync:
                        continue
                    seen_ldw = True
                new.append(ins)
            b.instructions[:] = new

    nc.compile()
    return nc


def host_prepare(obs, I, A, Bm):
    """Shard + precompute per-core device inputs and host bookkeeping."""
    import ml_dtypes
    bf16 = ml_dtypes.bfloat16
    import concourse.mybir as mybir
    em_np = mybir.dt.np(getattr(mybir.dt, EM_DTYPE))

    obs = np.asarray(obs)
    I64 = np.asarray(I, np.float64)
    A64 = np.asarray(A, np.float64)
    Bm64 = np.asarray(Bm, np.float64)

    pi = np.full(P, 1.0 / P)
    for _ in range(300):
        pi = pi @ A64
    f_sym = pi @ Bm64                                   # [E]
    Bmh = (Bm64 / f_sym[None, :]).astype(np.float32)    # folded emissions
    Bmh_em = Bmh.astype(em_np)

    A_bf = np.asarray(A, np.float32).astype(bf16)

    # device step index for every (segment, link), per stream kind
    t0d = np.array([_dseg_t0(s) for s in range(N_DSEG)])     # [60]
    stepd = t0d[:, None] + np.arange(LINKS)[None, :]         # [60, 94]
    t0t = np.array([_tseg_t0(p) for p in range(N_TSEG)])     # [62]
    stept = t0t[:, None] + np.arange(TLINKS)[None, :]        # [62, 47]

    in_maps = []
    book = []
    for c in range(N_CORES):
        ob = obs[c * BL:(c + 1) * BL]                   # [16, T]
        a0 = I64[:, None] * Bm64[:, ob[:, 0]]           # [S, 16]
        Z0 = a0.sum(0)
        alpha0 = (a0 / Z0).astype(np.float32).astype(bf16)

        m = {"amat": A_bf}
        for j in range(2):
            gs = np.arange(j * SEGS_D, (j + 1) * SEGS_D)
            sym = ob[:, stepd[gs]]                      # [16, segs, L]
            sym = sym.transpose(2, 1, 0).reshape(-1)
            m[f"emd{j}"] = np.ascontiguousarray(Bmh_em[:, sym])
            ai = np.full((P, W_D), 1.0 / P, np.float32)
            if j == 0:
                ai[:, :BL] = alpha0.astype(np.float32)
            m[f"ainitd{j}"] = ai.astype(bf16)

            gs = np.arange(j * SEGS_T, (j + 1) * SEGS_T)
            sym = ob[:, stept[gs]]
            sym = sym.transpose(2, 1, 0).reshape(-1)
            m[f"emt{j}"] = np.ascontiguousarray(Bmh_em[:, sym])
            ai = np.full((P, W_T), 1.0 / P, np.float32)
            m[f"ainitt{j}"] = ai.astype(bf16)
        in_maps.append(m)

        cnt = np.stack([(ob[:, 1:] == e).sum(1) for e in range(E_SYM)], 1)
        ll_base = np.log(Z0) + (cnt * np.log(f_sym)[None, :]).sum(1)  # [16]
        book.append(ll_base)
    return in_maps, book


def assemble_output(results, book):
    """Combine device outputs + host bookkeeping into loglik [128] f32."""
    out = np.empty(B_FULL, np.float64)
    for c in range(N_CORES):
        r = results[c]
        ll = book[c].copy()                             # [16]
        for j in range(2):
            cs_e = r[f"afind{j}"].astype(np.float64).reshape(
                P, SEGS_D, BL).sum(0)
            cs_w = r[f"stashd{j}"].astype(np.float64).reshape(
                P, SEGS_D, BL).sum(0)
            ll += np.log(cs_e).sum(0)
            lw = np.log(cs_w)
            if j == 0:
                lw = lw[1:]                             # seg 0: no warmup
            ll -= lw.sum(0)

            cs_e = r[f"afint{j}"].astype(np.float64).reshape(
                P, SEGS_T, BL).sum(0)
            cs_w = r[f"stasht{j}"].astype(np.float64).reshape(
                P, SEGS_T, BL).sum(0)
            ll += np.log(cs_e).sum(0) - np.log(cs_w).sum(0)
        out[c * BL:(c + 1) * BL] = ll
    return out.astype(np.float32)


_NC_CACHE = {}


def _get_nc():
    if "nc" not in _NC_CACHE:
        _NC_CACHE["nc"] = build_nc()
    return _NC_CACHE["nc"]


def kernel(obs, I, A, Bm):
    from concourse.bass_utils import run_bass_kernel_spmd

    nc = _get_nc()
    in_maps, book = host_prepare(obs, I, A, Bm)
    res = run_bass_kernel_spmd(nc, in_maps, core_ids=list(range(N_CORES)))
    return assemble_output(res.results, book)
